# revision 1
# baseline (speedup 1.0000x reference)
"""Trainium2 Bass kernel for DeepHedgingModel (LSTM scan, B=8192 T=512 F=4 H=32).

Strategy (pure data parallel over 8 cores, 1024 batch rows per core):

Per core, the 1024-row batch is split into 4 "bands" of 256 columns; band j
owns SBUF/PSUM partitions [32j, 32j+32).  All per-gate matmuls are M=32
tile_position matmuls whose outputs land inside the band, so every
elementwise LSTM op is partition-aligned and runs over all 4 bands in a
single [128, 256] instruction.

Gate-type column order in G is (g, i, f, o) so one Tanh covers cols 0:256
and one Sigmoid covers cols 256:1024.

z-vector per band = rows [d, x0..x3, ones] at partitions 32j..32j+6.
BatchNorm (inference affine) is folded into the stationary weights; the
d-feedback sigmoid is computed as tanh: sigma(y) = 0.5 + 0.5*tanh(y/2),
and the 0.5/0.5 affine is folded into the d-row weights + bias row, so the
raw tanh output is stored as the recurrent "d" row.  The output path
spreads d over partitions with a one-hot e_tau x d rank-1 matmul
accumulated in PSUM over 32 steps, then a DVE 32x32 stream-transpose moves
batch onto partitions; the 0.5+0.5x affine is applied once at the end.
"""

import sys
from contextlib import ExitStack

import numpy as np

sys.path.insert(0, "/opt/trn_rl_repo")

import concourse.tile as tile  # noqa: E402
from concourse import bacc, mybir  # noqa: E402

F32 = mybir.dt.float32
AF = mybir.ActivationFunctionType
ALU = mybir.AluOpType

EPS = 1e-5


# ----------------------------------------------------------------------------
# Config
# ----------------------------------------------------------------------------
class Cfg:
    def __init__(self, ncol=256, T=512, sblk=16, tau=32, nbands=4):
        self.ncol = ncol          # batch columns per band
        self.T = T                # timesteps
        self.sblk = sblk          # steps per x-DMA block
        self.tau = tau            # steps per d-spread epoch (<= 32)
        self.nbands = nbands      # 4 bands of 32 partitions
        self.B = nbands * ncol    # per-core batch
        assert T % sblk == 0 and T % tau == 0


FULL = Cfg()


# ----------------------------------------------------------------------------
# Host-side weight folding / input prep
# ----------------------------------------------------------------------------
def fold_params(p):
    """Return dict of numpy arrays for the SBUF-resident constants."""
    H = 32
    # gate-type order in G columns: (g, i, f, o); torch rows are (i, f, g, o)
    perm = np.concatenate([
        np.arange(2 * H, 3 * H),   # g
        np.arange(0, H),           # i
        np.arange(H, 2 * H),       # f
        np.arange(3 * H, 4 * H),   # o
    ])
    W_ih = p["W_ih"].astype(np.float64)
    W_hh = p["W_hh"].astype(np.float64)
    b_ih = p["b_ih"].astype(np.float64)
    b_hh = p["b_hh"].astype(np.float64)
    gam = p["bn_gamma"].astype(np.float64)
    bet = p["bn_beta"].astype(np.float64)
    mu = p["bn_mean"].astype(np.float64)
    var = p["bn_var"].astype(np.float64)
    a = gam / np.sqrt(var + EPS)          # [5]
    b_a = bet - mu * a                    # [5]

    Wx_eff = W_ih[:, :4] * a[None, :4]    # [128, 4]
    w_d_eff = W_ih[:, 4] * a[4]           # [128]
    C = b_a @ W_ih.T + b_ih + b_hh        # [128]
    # recurrence stores t_y = tanh(y/2);  d = 0.5 + 0.5 t_y
    d_row = 0.5 * w_d_eff                 # weight on stored t_y
    C = C + 0.5 * w_d_eff                 # constant part of d contribution

    # scale the tanh-gate (g) rows?  no: direct tanh table is used.
    out = {}
    # Whh_sb [128, 128]: [32j+k, 32g+m] = W_hh[perm[32g+m], k]
    Whh_band = W_hh[perm, :].T            # [k 32, 128 gates(permuted)]
    out["whh"] = np.tile(Whh_band[:, :], (4, 1)).astype(np.float32)  # [128,128]
    # Wx1d_sb [128, 128]: band rows r: 0=d_row, 1..4=x, 5=ones(C)
    wx1d_band = np.zeros((32, 128))
    wx1d_band[0, :] = d_row[perm]
    wx1d_band[1:5, :] = Wx_eff[perm, :].T
    wx1d_band[5, :] = C[perm]
    out["wx1d"] = np.tile(wx1d_band, (4, 1)).astype(np.float32)      # [128,128]
    # Wm1_sb [128, 32]: [32j+k, m] = W1[m, k]
    W1 = p["W1"].astype(np.float64)
    out["wm1"] = np.tile(W1.T, (4, 1)).astype(np.float32)            # [128,32]
    # Wm2_sb [128, 1]: [32j+k, 0] = W2[0, k]
    W2 = p["W2"].astype(np.float64)
    out["wm2"] = np.tile(W2.T, (4, 1)).astype(np.float32)            # [128,1]
    # per-partition b1 bias column [128,1]
    out["b1col"] = np.tile(p["b1"].astype(np.float32)[:, None], (4, 1))
    out["b2half"] = float(0.5 * p["b2"].astype(np.float64)[0])
    return out


def make_eye(cfg):
    """E_sb [128, tau*32]: row 32j holds one-hot e_tau blocks."""
    E = np.zeros((128, cfg.tau * 32), np.float32)
    for t in range(cfg.tau):
        E[::32, t * 32 + t] = 1.0
    return E


def prep_x(x_core, cfg):
    """x_core [B, T, 4] -> xprep [T/sblk, nbands, 5, sblk, ncol] (ones in row 4)."""
    B, T, F = x_core.shape
    nb, nc_, sb = cfg.nbands, cfg.ncol, cfg.sblk
    xp = np.empty((T // sb, nb, 5, sb, nc_), np.float32)
    # x_core[b, t, f] with b = j*ncol + n
    xr = x_core.reshape(nb, nc_, T // sb, sb, F)
    xp[:, :, :4] = xr.transpose(2, 0, 4, 3, 1)  # [blk, band, f, s, n]
    xp[:, :, 4] = 1.0
    return xp


# ----------------------------------------------------------------------------
# Kernel body
# ----------------------------------------------------------------------------
def build_kernel(nc, cfg, use_strided=False, time_mode=False):
    """Declare DRAM I/O and emit the TileContext program.

    time_mode=True shrinks xprep to one block that every step re-reads —
    wrong math, identical instruction stream — to measure device time
    without the axon per-call input-transfer cost.
    """
    N = cfg.ncol
    T, SB, TAU = cfg.T, cfg.sblk, cfg.tau

    nxblk = 1 if time_mode else T // SB
    d_x = nc.dram_tensor("xprep", [nxblk, 4, 5, SB, N], F32, kind="ExternalInput")
    d_whh = nc.dram_tensor("whh", [128, 128], F32, kind="ExternalInput")
    d_wx1d = nc.dram_tensor("wx1d", [128, 128], F32, kind="ExternalInput")
    d_wm1 = nc.dram_tensor("wm1", [128, 32], F32, kind="ExternalInput")
    d_wm2 = nc.dram_tensor("wm2", [128, 1], F32, kind="ExternalInput")
    d_b1 = nc.dram_tensor("b1col", [128, 1], F32, kind="ExternalInput")
    d_b2h = nc.dram_tensor("b2half", [128, 1], F32, kind="ExternalInput")
    # raw tanh(y/2) history, dumped per x-block; host does transpose + affine
    d_out = nc.dram_tensor("d_scratch", [T // SB + 1, 4, SB * N], F32,
                           kind="ExternalOutput")

    with tile.TileContext(nc) as tc, ExitStack() as ctx:
        wp = ctx.enter_context(tc.tile_pool(name="weights", bufs=1))
        zp = ctx.enter_context(tc.tile_pool(name="zb", bufs=3))
        sp = ctx.enter_context(tc.tile_pool(name="sgate", bufs=2))
        hp = ctx.enter_context(tc.tile_pool(name="hstate", bufs=2))
        cp = ctx.enter_context(tc.tile_pool(name="cstate", bufs=1))
        tp = ctx.enter_context(tc.tile_pool(name="tmp", bufs=2))
        pg = ctx.enter_context(tc.tile_pool(name="psum_g", bufs=2, space="PSUM"))
        pm = ctx.enter_context(tc.tile_pool(name="psum_m", bufs=1, space="PSUM"))
        py = ctx.enter_context(tc.tile_pool(name="psum_y", bufs=1, space="PSUM"))

        # --- constants into SBUF
        whh = wp.tile([128, 128], F32)
        nc.sync.dma_start(whh[:], d_whh[:])
        wx1d = wp.tile([128, 128], F32)
        nc.sync.dma_start(wx1d[:], d_wx1d[:])
        wm1 = wp.tile([128, 32], F32)
        nc.sync.dma_start(wm1[:], d_wm1[:])
        wm2 = wp.tile([128, 1], F32)
        nc.sync.dma_start(wm2[:], d_wm2[:])
        b1c = wp.tile([128, 1], F32)
        nc.sync.dma_start(b1c[:], d_b1[:])
        b2h = wp.tile([128, 1], F32)
        nc.sync.dma_start(b2h[:], d_b2h[:])

        # --- state
        cst = cp.tile([128, N], F32)
        nc.vector.memset(cst[:], 0.0)
        hst = cp.tile([128, N], F32)
        nc.vector.memset(hst[:], 0.0)
        dtail = cp.tile([128, N], F32)

        # --- x block tiles, created on demand (prefetched one block early)
        zb_tiles = {}

        def get_zb(blk):
            if blk not in zb_tiles:
                zt = zp.tile([128, SB * N], F32, tag="zb")
                xblk = 0 if time_mode else blk
                for j in range(4):
                    nc.sync.dma_start(zt[32 * j + 1 : 32 * j + 6, :], d_x[xblk, j])
                zb_tiles[blk] = zt
                if len(zb_tiles) > 3:
                    del zb_tiles[min(zb_tiles)]
            return zb_tiles[blk]

        z0 = get_zb(0)
        # d(t=0) stored value: tanh form of d=0 is -1
        z0v = z0[:].rearrange("(a p) (s n) -> a p s n", p=32, n=N)
        if use_strided:
            nc.vector.memset(z0v[:, 0, 0], -1.0)
        else:
            for j in range(4):
                nc.vector.memset(z0[32 * j : 32 * j + 1, 0:N], -1.0)

        for t in range(T):
            blk, s = divmod(t, SB)
            zbt = get_zb(blk)
            if s == 0 and blk + 1 < T // SB:
                get_zb(blk + 1)  # prefetch next x block
            if t + 1 < T:
                nblk, ns = divmod(t + 1, SB)
                zbn, nscol = get_zb(nblk), ns
            else:
                zbn, nscol = dtail, 0

            # Two independent column phases: phase ph owns batch columns
            # [c0, c0+W) of every band. The two recurrences share no data,
            # so the scheduler interleaves them and fills chain stalls.
            W = N // 2
            for ph in range(2):
                c0 = ph * W
                cs = slice(s * N + c0, s * N + c0 + W)       # zbt cols
                ns_ = slice(nscol * N + c0, nscol * N + c0 + W)  # zbn cols

                # ---- gates: G[128, 4W] cols (g,i,f,o)
                G = pg.tile([128, 4 * W], F32, tag=f"G{ph}")
                for j in range(4):
                    r = 32 * j
                    for g in range(4):
                        gc = slice(g * W, (g + 1) * W)
                        nc.tensor.matmul(
                            G[r : r + 32, gc],
                            wx1d[r : r + 6, g * 32 : g * 32 + 32],
                            zbt[r : r + 6, cs],
                            start=True, stop=False, tile_position=(r, r),
                        )
                        nc.tensor.matmul(
                            G[r : r + 32, gc],
                            whh[r : r + 32, g * 32 : g * 32 + 32],
                            hst[r : r + 32, c0 : c0 + W],
                            start=False, stop=True, tile_position=(r, r),
                        )

                # ---- activations on gates
                S = sp.tile([128, 4 * W], F32, tag=f"S{ph}")
                nc.scalar.activation(S[:, 0:W], G[:, 0:W], AF.Tanh)
                nc.scalar.activation(S[:, W : 4 * W], G[:, W : 4 * W],
                                     AF.Sigmoid)

                # ---- c, h update (cols: 0:W=tg, W:2W=si, 2W:3W=sf, 3W:4W=so)
                q = tp.tile([128, W], F32, tag=f"q{ph}")
                nc.vector.tensor_mul(q[:], S[:, 2 * W : 3 * W],
                                     cst[:, c0 : c0 + W])
                pp = tp.tile([128, W], F32, tag=f"p{ph}")
                nc.vector.tensor_mul(pp[:], S[:, W : 2 * W], S[:, 0:W])
                nc.vector.tensor_add(cst[:, c0 : c0 + W], q[:], pp[:])
                th = tp.tile([128, W], F32, tag=f"th{ph}")
                nc.scalar.activation(th[:], cst[:, c0 : c0 + W], AF.Tanh)
                nc.vector.tensor_mul(hst[:, c0 : c0 + W],
                                     S[:, 3 * W : 4 * W], th[:])

                # ---- decision MLP
                M1 = pm.tile([128, W], F32, tag=f"M1{ph}")
                for j in range(4):
                    r = 32 * j
                    nc.tensor.matmul(M1[r : r + 32, :], wm1[r : r + 32, :],
                                     hst[r : r + 32, c0 : c0 + W],
                                     start=True, stop=True,
                                     tile_position=(r, r),
                                     skip_group_check=True)
                R = tp.tile([128, W], F32, tag=f"R{ph}")
                nc.vector.tensor_scalar(R[:], M1[:], b1c[:, 0:1], 0.0,
                                        ALU.add, ALU.max)
                Y = py.tile([128, W], F32, tag=f"Y{ph}")
                for j in range(4):
                    r = 32 * j
                    nc.tensor.matmul(Y[r : r + 1, :], wm2[r : r + 32, 0:1],
                                     R[r : r + 32, :], start=True, stop=True,
                                     tile_position=(r, r),
                                     skip_group_check=True)

                # ---- d = tanh(0.5 y + 0.5 b2)  (stored tanh form)
                for j in range(4):
                    r = 32 * j
                    nc.scalar.activation(
                        zbn[r : r + 1, ns_],
                        Y[r : r + 1, :], AF.Tanh,
                        bias=b2h[r : r + 1, 0:1], scale=0.5)

            # ---- dump this block's d-rows once its last column is written
            if s == SB - 1:
                for j in range(4):
                    nc.sync.dma_start(d_out[blk, j].unsqueeze(0),
                                      zbt[32 * j : 32 * j + 1, :])

        # final d (t = T-1) lives in dtail column 0
        for j in range(4):
            nc.sync.dma_start(d_out[T // SB, j, 0:N].unsqueeze(0),
                              dtail[32 * j : 32 * j + 1, 0:N])

    return d_out


def gather_out(scratch, cfg):
    """d_scratch [T/SB+1, 4, SB*N] (tanh form, shifted by one) -> [B, T]."""
    T, SB, N = cfg.T, cfg.sblk, cfg.ncol
    sc = np.asarray(scratch).reshape(T // SB + 1, 4, SB, N)
    seq = sc.transpose(1, 3, 0, 2).reshape(4, N, (T // SB + 1) * SB)
    vals = seq[:, :, 1 : T + 1]                   # drop the t=-1 init slot
    return (0.5 + 0.5 * vals).reshape(cfg.B, T).astype(np.float32)


# ----------------------------------------------------------------------------
# numpy reference of the exact kernel math (for mini-tests)
# ----------------------------------------------------------------------------
def numpy_model(x, params):
    """x [B, T, 4] -> [B, T] float32, same math as reference()."""
    import jax
    import jax.numpy as jnp
    B, T, F = x.shape
    H = params["W_hh"].shape[1]
    inv_std = 1.0 / np.sqrt(params["bn_var"] + EPS)

    h = np.zeros((B, H), np.float32)
    c = np.zeros((B, H), np.float32)
    d = np.zeros((B, 1), np.float32)
    outs = np.zeros((B, T), np.float32)
    sig = lambda v: 1.0 / (1.0 + np.exp(-v))
    for t in range(T):
        z = np.concatenate([x[:, t], d], 1)
        z = (z - params["bn_mean"]) * inv_std * params["bn_gamma"] + params["bn_beta"]
        gates = z @ params["W_ih"].T + params["b_ih"] + h @ params["W_hh"].T + params["b_hh"]
        i, f, g, o = np.split(gates, 4, 1)
        c = sig(f) * c + sig(i) * np.tanh(g)
        h = sig(o) * np.tanh(c)
        d = sig(np.maximum(h @ params["W1"].T + params["b1"], 0) @ params["W2"].T + params["b2"])
        outs[:, t] = d[:, 0]
    return outs


# ----------------------------------------------------------------------------
# Entry point
# ----------------------------------------------------------------------------
_CACHE = {}


def _get_compiled():
    if "nc" not in _CACHE:
        nc = bacc.Bacc("TRN2", target_bir_lowering=False, debug=False)
        build_kernel(nc, FULL)
        nc.compile()
        _CACHE["nc"] = nc
    return _CACHE["nc"]


def kernel(**inputs):
    from concourse.bass_utils import run_bass_kernel_spmd

    x = np.asarray(inputs["x"], np.float32)
    B, T, F = x.shape
    ncores = 8
    bc = B // ncores
    folded = fold_params(inputs)
    b2h = np.full((128, 1), folded["b2half"], np.float32)

    nc = _get_compiled()
    in_maps = []
    for c in range(ncores):
        m = {
            "xprep": prep_x(x[c * bc : (c + 1) * bc], FULL),
            "whh": folded["whh"],
            "wx1d": folded["wx1d"],
            "wm1": folded["wm1"],
            "wm2": folded["wm2"],
            "b1col": folded["b1col"],
            "b2half": b2h,
        }
        in_maps.append(m)

    res = run_bass_kernel_spmd(nc, in_maps, list(range(ncores)))
    outs = [gather_out(res.results[c]["d_scratch"], FULL) for c in range(ncores)]
    return np.concatenate(outs, 0)[:, :, None].astype(np.float32)



# revision 3
# speedup vs baseline: 1.3269x; 1.3269x over previous
"""Trainium2 Bass kernel for DeepHedgingModel (LSTM scan, B=8192 T=512 F=4 H=32).

Data parallel over 8 cores (1024 batch rows each).  Per core, the batch is
split into 4 "bands" of 256 columns; band j owns SBUF/PSUM partitions
[32j, 32j+32).  All matmuls are 32x32 tile_position matmuls in bf16 (single
HW pass, vs 2 LOW/HIGH passes for fp32), elementwise ops run over all 4
bands in single [128, W] instructions.

All four gates take a single Tanh activation: sigmoid gates (i, f, o) have
their weights pre-halved so sigmoid(y) = 0.5 + 0.5*tanh(y/2); the affine is
folded into fused scalar_tensor_tensor DVE ops using a doubled cell state
C2 = 2c and doubled hidden state h2 = 2h (the 0.5 is folded into W_hh/W1).

    gates:  G = Wz^T z + Wd^T t_d + Whh^T h2        (PSUM, fp32)
    T     = tanh(G)                                  one ACT over [128, 4W]
    u     = (T_f + 1) * C2                           stt
    p     = (T_i + 1) * T_g                          stt
    C2'   = 0.5*u + p                                stt
    th    = tanh(0.5*C2')                            ACT
    h2'   = (T_o + 1) * th                           stt      [bf16]
    M1    = (0.5 W1)^T h2'                           matmul
    R     = relu(M1 + b1)                            tensor_scalar [bf16]
    y     = broadcast-W2^T R                         one K=128 matmul whose
                                                     block-diag stationary
                                                     broadcasts band j's y to
                                                     all of strip j
    t_d   = tanh(0.5*y + 0.5*b2)                     ACT into Dblk [bf16]

Dblk is a per-16-step staging tile that both feeds the next step's d-matmul
(row 32j of strip j) and is DMA'd out as the kernel output (tanh form; host
applies 0.5 + 0.5*t).
"""

import sys
from contextlib import ExitStack

import numpy as np

sys.path.insert(0, "/opt/trn_rl_repo")

import concourse.tile as tile  # noqa: E402
from concourse import bacc, mybir  # noqa: E402

F32 = mybir.dt.float32
BF16 = mybir.dt.bfloat16
AF = mybir.ActivationFunctionType
ALU = mybir.AluOpType

EPS = 1e-5
H = 32


# ----------------------------------------------------------------------------
# Config
# ----------------------------------------------------------------------------
class Cfg:
    def __init__(self, ncol=256, T=512, sblk=16, nchunks=2):
        self.ncol = ncol            # batch columns per band
        self.T = T                  # timesteps
        self.sblk = sblk            # steps per x-DMA block / out-DMA block
        self.nchunks = nchunks      # column chunks per step (pipelining)
        self.W = ncol // nchunks    # columns per chunk
        self.B = 4 * ncol           # per-core batch
        assert T % sblk == 0 and ncol % nchunks == 0


FULL = Cfg()


# ----------------------------------------------------------------------------
# Host-side weight folding / input prep
# ----------------------------------------------------------------------------
def fold_params(p):
    """Return dict of numpy arrays for the SBUF-resident constants (bf16/f32).

    Gate-type order in G columns is (g, i, f, o); torch rows are (i, f, g, o).
    Sigmoid gates (i, f, o) are pre-halved (tanh half-angle identity), and the
    recurrent weights carry an extra 0.5 because the stored h2 = 2h.
    """
    import ml_dtypes

    W_ih = p["W_ih"].astype(np.float64)
    W_hh = p["W_hh"].astype(np.float64)
    b_ih = p["b_ih"].astype(np.float64)
    b_hh = p["b_hh"].astype(np.float64)
    gam = p["bn_gamma"].astype(np.float64)
    bet = p["bn_beta"].astype(np.float64)
    mu = p["bn_mean"].astype(np.float64)
    var = p["bn_var"].astype(np.float64)

    a = gam / np.sqrt(var + EPS)          # [5]
    b_a = bet - mu * a                    # [5]
    Wx_eff = W_ih[:, :4] * a[None, :4]    # [128, 4]
    w_d_eff = W_ih[:, 4] * a[4]           # [128]
    C = b_a @ W_ih.T + b_ih + b_hh + 0.5 * w_d_eff   # [128]
    d_row = 0.5 * w_d_eff                 # weight on stored t_d

    # permutation: G column group order (g, i, f, o); 0.5 gate scale on i,f,o
    perm = np.concatenate([
        np.arange(2 * H, 3 * H),   # g
        np.arange(0, H),           # i
        np.arange(H, 2 * H),       # f
        np.arange(3 * H, 4 * H),   # o
    ])
    gscale = np.concatenate([np.ones(H), np.full(3 * H, 0.5)])  # [128]

    bf = ml_dtypes.bfloat16
    out = {}
    # wz_band [32, 128]: rows 0..3 = x feats, row 4 = C (ones row)
    wz = np.zeros((32, 128))
    wz[0:4, :] = Wx_eff[perm, :].T * gscale[None, :]
    wz[4, :] = C[perm] * gscale
    out["wz"] = np.tile(wz, (4, 1)).astype(bf)                   # [128,128]
    # wd_band [32, 128]: row 0 = d feedback weights on t_d
    wd = np.zeros((32, 128))
    wd[0, :] = d_row[perm] * gscale
    out["wd"] = np.tile(wd, (4, 1)).astype(bf)                   # [128,128]
    # whh_band [32, 128]: [k, 32g+m] = 0.5 * gscale * W_hh[perm[32g+m], k]
    whh = 0.5 * W_hh[perm, :].T * gscale[None, :]
    out["whh"] = np.tile(whh, (4, 1)).astype(bf)                 # [128,128]
    # wm1 [128, 32]: 0.5 * W1^T  (h2 = 2h fold)
    W1 = p["W1"].astype(np.float64)
    out["wm1"] = np.tile(0.5 * W1.T, (4, 1)).astype(bf)          # [128,32]
    # wy [128, 128]: block-diag broadcast of W2: wy[32j+k, c] = W2[0,k] iff c//32==j
    W2 = p["W2"].astype(np.float64)
    wy = np.zeros((128, 128))
    for j in range(4):
        wy[32 * j : 32 * j + 32, 32 * j : 32 * j + 32] = W2[0][:, None]
    out["wy"] = wy.astype(bf)                                    # [128,128]
    out["b1col"] = np.tile(p["b1"].astype(np.float32)[:, None], (4, 1))
    out["b2h"] = np.full((128, 1), 0.5 * p["b2"].astype(np.float64)[0],
                         np.float32)
    return out


def prep_x(x_core, cfg):
    """x_core [B, T, 4] f32 -> xprep [T/sblk, 4, 5, sblk, ncol] bf16.

    Row 4 of the feature axis is the constant ones row (C's matmul input).
    """
    import ml_dtypes

    B, T, F = x_core.shape
    nc_, sb = cfg.ncol, cfg.sblk
    xp = np.empty((T // sb, 4, 5, sb, nc_), ml_dtypes.bfloat16)
    xr = x_core.reshape(4, nc_, T // sb, sb, F)
    xp[:, :, :4] = xr.transpose(2, 0, 4, 3, 1).astype(ml_dtypes.bfloat16)
    xp[:, :, 4] = ml_dtypes.bfloat16(1.0)
    return xp


# ----------------------------------------------------------------------------
# Kernel body
# ----------------------------------------------------------------------------
def build_kernel(nc, cfg):
    N, T, SB, NCH, W = cfg.ncol, cfg.T, cfg.sblk, cfg.nchunks, cfg.W

    d_x = nc.dram_tensor("xprep", [T // SB, 4, 5, SB, N], BF16,
                         kind="ExternalInput")
    d_wz = nc.dram_tensor("wz", [128, 128], BF16, kind="ExternalInput")
    d_wd = nc.dram_tensor("wd", [128, 128], BF16, kind="ExternalInput")
    d_whh = nc.dram_tensor("whh", [128, 128], BF16, kind="ExternalInput")
    d_wm1 = nc.dram_tensor("wm1", [128, 32], BF16, kind="ExternalInput")
    d_wy = nc.dram_tensor("wy", [128, 128], BF16, kind="ExternalInput")
    d_b1 = nc.dram_tensor("b1col", [128, 1], F32, kind="ExternalInput")
    d_b2h = nc.dram_tensor("b2h", [128, 1], F32, kind="ExternalInput")
    # t_d history in tanh form; host applies 0.5 + 0.5*t
    d_out = nc.dram_tensor("d_scratch", [T // SB, 4, SB * N], BF16,
                           kind="ExternalOutput")

    with tile.TileContext(nc) as tc, ExitStack() as ctx:
        wp = ctx.enter_context(tc.tile_pool(name="weights", bufs=1))
        zp = ctx.enter_context(tc.tile_pool(name="zb", bufs=3))
        dp = ctx.enter_context(tc.tile_pool(name="dblk", bufs=2))
        sp = ctx.enter_context(tc.tile_pool(name="work", bufs=2))
        cp = ctx.enter_context(tc.tile_pool(name="state", bufs=2))
        ip = ctx.enter_context(tc.tile_pool(name="init", bufs=1))
        pg = ctx.enter_context(tc.tile_pool(name="psum_g", bufs=2, space="PSUM"))
        pm = ctx.enter_context(tc.tile_pool(name="psum_m", bufs=1, space="PSUM"))
        py = ctx.enter_context(tc.tile_pool(name="psum_y", bufs=1, space="PSUM"))

        # --- constants into SBUF
        wz = wp.tile([128, 128], BF16)
        nc.sync.dma_start(wz[:], d_wz[:])
        wd = wp.tile([128, 128], BF16)
        nc.sync.dma_start(wd[:], d_wd[:])
        whh = wp.tile([128, 128], BF16)
        nc.sync.dma_start(whh[:], d_whh[:])
        wm1 = wp.tile([128, 32], BF16)
        nc.sync.dma_start(wm1[:], d_wm1[:])
        wy = wp.tile([128, 128], BF16)
        nc.sync.dma_start(wy[:], d_wy[:])
        b1c = wp.tile([128, 1], F32)
        nc.sync.dma_start(b1c[:], d_b1[:])
        b2h = wp.tile([128, 1], F32)
        nc.sync.dma_start(b2h[:], d_b2h[:])

        # --- state (per chunk): C2 = 2c fp32, h2 = 2h bf16, t_d in Dblk
        h2 = [None] * NCH
        C2 = [None] * NCH
        for ph in range(NCH):
            h2i = ip.tile([128, W], BF16, tag=f"h2i{ph}")
            nc.vector.memset(h2i[:], 0.0)
            c2i = ip.tile([128, W], F32, tag=f"c2i{ph}")
            nc.vector.memset(c2i[:], 0.0)
            h2[ph], C2[ph] = h2i, c2i
        dinit = ip.tile([128, N], BF16)
        nc.vector.memset(dinit[:], -1.0)   # t_d of d=0

        # --- x block tiles, created on demand (prefetched one block early)
        zb_tiles = {}

        def get_zb(blk):
            if blk not in zb_tiles:
                zt = zp.tile([128, SB * N], BF16, tag="zb")
                for j in range(4):
                    nc.sync.dma_start(zt[32 * j : 32 * j + 5, :], d_x[blk, j])
                zb_tiles[blk] = zt
                if len(zb_tiles) > 3:
                    del zb_tiles[min(zb_tiles)]
            return zb_tiles[blk]

        get_zb(0)
        dblk_cur = None
        dblk_prev = None

        for t in range(T):
            blk, s = divmod(t, SB)
            zbt = get_zb(blk)
            if s == 0:
                if blk + 1 < T // SB:
                    get_zb(blk + 1)  # prefetch next x block
                dblk_prev = dblk_cur
                dblk_cur = dp.tile([128, SB * N], BF16, tag="dblk")

            for ph in range(NCH):
                c0 = ph * W
                zc = slice(s * N + c0, s * N + c0 + W)       # z/Dblk cols

                # previous t_d source
                if t == 0:
                    dsrc, dc = dinit, slice(c0, c0 + W)
                elif s == 0:
                    dsrc, dc = dblk_prev, slice((SB - 1) * N + c0,
                                                (SB - 1) * N + c0 + W)
                else:
                    dsrc, dc = dblk_cur, slice((s - 1) * N + c0,
                                               (s - 1) * N + c0 + W)

                # ---- gates: G[128, 4W] cols (g, i, f, o)
                G = pg.tile([128, 4 * W], F32, tag=f"G{ph}")
                for j in range(4):
                    r = 32 * j
                    for g in range(4):
                        gc = slice(g * W, (g + 1) * W)
                        wc = slice(g * 32, g * 32 + 32)
                        nc.tensor.matmul(
                            G[r : r + 32, gc], whh[r : r + 32, wc],
                            h2[ph][r : r + 32, :],
                            start=True, stop=False, tile_position=(r, r))
                        nc.tensor.matmul(
                            G[r : r + 32, gc], wz[r : r + 5, wc],
                            zbt[r : r + 5, zc],
                            start=False, stop=False, tile_position=(r, r))
                        nc.tensor.matmul(
                            G[r : r + 32, gc], wd[r : r + 1, wc],
                            dsrc[r : r + 1, dc],
                            start=False, stop=True, tile_position=(r, r))

                # ---- one tanh over all gates
                Tt = sp.tile([128, 4 * W], F32, tag=f"T{ph}")
                nc.scalar.activation(Tt[:], G[:], AF.Tanh)

                # ---- C2' = 0.5*(T_f+1)*C2 + (T_i+1)*T_g
                u = sp.tile([128, W], F32, tag=f"u{ph}")
                nc.vector.scalar_tensor_tensor(
                    u[:], Tt[:, 2 * W : 3 * W], 1.0, C2[ph][:],
                    ALU.add, ALU.mult)
                pp_ = sp.tile([128, W], F32, tag=f"p{ph}")
                nc.vector.scalar_tensor_tensor(
                    pp_[:], Tt[:, W : 2 * W], 1.0, Tt[:, 0:W],
                    ALU.add, ALU.mult)
                C2n = cp.tile([128, W], F32, tag=f"C2{ph}")
                nc.vector.scalar_tensor_tensor(
                    C2n[:], u[:], 0.5, pp_[:], ALU.mult, ALU.add)
                C2[ph] = C2n

                # ---- h2' = (T_o+1) * tanh(c)
                th = sp.tile([128, W], F32, tag=f"th{ph}")
                nc.scalar.activation(th[:], C2n[:], AF.Tanh, scale=0.5)
                h2n = cp.tile([128, W], BF16, tag=f"h2{ph}")
                nc.vector.scalar_tensor_tensor(
                    h2n[:], Tt[:, 3 * W : 4 * W], 1.0, th[:],
                    ALU.add, ALU.mult)
                h2[ph] = h2n

                # ---- decision MLP
                M1 = pm.tile([128, W], F32, tag=f"M1{ph}")
                for j in range(4):
                    r = 32 * j
                    nc.tensor.matmul(M1[r : r + 32, :], wm1[r : r + 32, :],
                                     h2n[r : r + 32, :], start=True, stop=True,
                                     tile_position=(r, r),
                                     skip_group_check=True)
                R = sp.tile([128, W], BF16, tag=f"R{ph}")
                nc.vector.tensor_scalar(R[:], M1[:], b1c[:, 0:1], 0.0,
                                        ALU.add, ALU.max)
                # y broadcast to strips: Y[32j+k, n] = y_band_j[n]
                Y = py.tile([128, W], F32, tag=f"Y{ph}")
                nc.tensor.matmul(Y[:], wy[:], R[:], start=True, stop=True,
                                 skip_group_check=True)

                # ---- t_d = tanh(0.5 y + 0.5 b2) into Dblk column slice
                nc.scalar.activation(dblk_cur[:, zc], Y[:], AF.Tanh,
                                     bias=b2h[:, 0:1], scale=0.5)

            # ---- dump this block's t_d rows once its last column is written
            if s == SB - 1:
                for j in range(4):
                    nc.sync.dma_start(d_out[blk, j].unsqueeze(0),
                                      dblk_cur[32 * j : 32 * j + 1, :])

    return d_out


def gather_out(scratch, cfg):
    """d_scratch [T/SB, 4, SB*N] bf16 (tanh form) -> [B, T] float32."""
    T, SB, N = cfg.T, cfg.sblk, cfg.ncol
    sc = np.asarray(scratch).astype(np.float32).reshape(T // SB, 4, SB, N)
    seq = sc.transpose(1, 3, 0, 2).reshape(cfg.B, T)
    return (0.5 + 0.5 * seq).astype(np.float32)


# ----------------------------------------------------------------------------
# Entry point
# ----------------------------------------------------------------------------
_CACHE = {}


def _get_compiled():
    if "nc" not in _CACHE:
        nc = bacc.Bacc("TRN2", target_bir_lowering=False, debug=False)
        build_kernel(nc, FULL)
        nc.compile()
        _CACHE["nc"] = nc
    return _CACHE["nc"]


def kernel(**inputs):
    from concourse.bass_utils import run_bass_kernel_spmd

    x = np.asarray(inputs["x"], np.float32)
    B, T, F = x.shape
    ncores = 8
    bc = B // ncores
    folded = fold_params(inputs)

    nc = _get_compiled()
    in_maps = []
    for c in range(ncores):
        m = {"xprep": prep_x(x[c * bc : (c + 1) * bc], FULL)}
        m.update(folded)
        in_maps.append(m)

    res = run_bass_kernel_spmd(nc, in_maps, list(range(ncores)))
    outs = [gather_out(res.results[c]["d_scratch"], FULL) for c in range(ncores)]
    return np.concatenate(outs, 0)[:, :, None].astype(np.float32)
